# Initial kernel scaffold
#
"""Trainium2 Bass kernel for retrieval_knn (nn_CLI_63702954934481).

Data-parallel over batch B=8: one batch per NeuronCore. Per core:
integer-exact cdist keys via PE matmul (bf16-encoded shifted coords),
group-max pyramid selection (DVE grouped reduce + hw top-8 max/max_index),
exact candidate re-ranking with full (d2, index) tie-breaking,
indirect-DMA gathers, weighted sum, concat. Self-contained.
"""
import sys

if "/opt/trn_rl_repo" not in sys.path:
    sys.path.insert(0, "/opt/trn_rl_repo")

import numpy as np
from concourse import bacc
from concourse.bass_utils import run_bass_kernel_spmd

from contextlib import ExitStack

import concourse.bass as bass
import concourse.mybir as mybir
import concourse.tile as tile
from concourse._compat import with_exitstack
from concourse.alu_op_type import AluOpType

P = 128
NA = 4096
NB = 4096
D = 64
K = 3
NT = NA // P
G = 16
NG = NB // G      # 256 groups
NC = K * G        # 48 candidates
NIDX = P * K      # 384 gather rows per tile
WS = NIDX // 16   # 24 wrapped slots
FS = 128.0
R = 0.5

f32 = mybir.dt.float32
i32 = mybir.dt.int32
u32 = mybir.dt.uint32
i16 = mybir.dt.int16
bf16 = mybir.dt.bfloat16
Act = mybir.ActivationFunctionType
X = mybir.AxisListType.X


@with_exitstack
def knn_tile_kernel3(ctx: ExitStack, tc: tile.TileContext, outs, ins):
    nc = tc.nc
    a_feats, b_feats, a_coords, b_coords = ins
    out = outs[0]

    prep_pool = ctx.enter_context(tc.tile_pool(name="prep", bufs=1))
    row_pool = ctx.enter_context(tc.tile_pool(name="rows", bufs=1))
    psum_pool = ctx.enter_context(tc.tile_pool(name="ps", bufs=2, space="PSUM"))
    sp = ctx.enter_context(tc.tile_pool(name="small", bufs=6))

    # ---------------- Phase 0 ----------------
    prep_dram = nc.dram_tensor("prep_scratch", [10, NB], bf16)
    qb_dram = nc.dram_tensor("qb_scratch", [NB, 4], f32)

    def load_q(coords, base_row, scaled_by_2):
        tag = f"cp{base_row}"
        c = prep_pool.tile([P, NT * 3], i32, name=f"c_{tag}")
        nc.sync.dma_start(c[:], coords.rearrange("(t p) d -> p t d", p=P))
        q_i = prep_pool.tile([P, NT * 3], i32, name=f"qi_{tag}")
        nc.vector.tensor_scalar(q_i[:], c[:], 4, None, op0=AluOpType.arith_shift_right)
        q_f = prep_pool.tile([P, NT * 3], f32, name=f"qf_{tag}")
        nc.vector.tensor_copy(q_f[:], q_i[:])
        q_bf = prep_pool.tile([P, NT * 3], bf16, name=f"qbf_{tag}")
        if scaled_by_2:
            nc.vector.tensor_scalar(q_bf[:], q_f[:], 2.0, None, op0=AluOpType.mult)
        else:
            nc.vector.tensor_copy(q_bf[:], q_f[:])
        for d in range(3):
            src = q_bf[:].rearrange("p (t d) -> p t d", d=3)[:, :, d]
            dst = prep_dram[base_row + d].rearrange("(t p) -> p t", p=P)
            nc.sync.dma_start(dst, src)
        return q_f

    qa_f = load_q(a_coords, 0, True)
    qb_f = load_q(b_coords, 3, False)

    sqa = prep_pool.tile([P, NT * 3], f32)
    nc.vector.tensor_tensor(out=sqa[:], in0=qa_f[:], in1=qa_f[:], op=AluOpType.mult)
    a2 = prep_pool.tile([P, NT], f32)
    nc.vector.tensor_reduce(out=a2[:], in_=sqa[:].rearrange("p (t d) -> p t d", d=3),
                            axis=X, op=AluOpType.add)

    sqb = prep_pool.tile([P, NT * 3], f32)
    nc.vector.tensor_tensor(out=sqb[:], in0=qb_f[:], in1=qb_f[:], op=AluOpType.mult)
    b2 = prep_pool.tile([P, NT], f32)
    nc.vector.tensor_reduce(out=b2[:], in_=sqb[:].rearrange("p (t d) -> p t d", d=3),
                            axis=X, op=AluOpType.add)
    nc.sync.dma_start(
        qb_dram.rearrange("(t p) d -> p t d", p=P)[:, :, 0:3],
        qb_f[:].rearrange("p (t d) -> p t d", d=3))
    nc.sync.dma_start(qb_dram.rearrange("(t p) d -> p t d", p=P)[:, :, 3], b2[:])

    b2_i = prep_pool.tile([P, NT], i32)
    nc.vector.tensor_copy(b2_i[:], b2[:])
    hb_i = prep_pool.tile([P, NT], i32)
    nc.vector.tensor_scalar(hb_i[:], b2_i[:], 8, None, op0=AluOpType.arith_shift_right)
    lb_i = prep_pool.tile([P, NT], i32)
    nc.vector.tensor_scalar(lb_i[:], b2_i[:], 255, None, op0=AluOpType.bitwise_and)
    for nm, src_i, row in (("hb", hb_i, 6), ("lb", lb_i, 7)):
        t_f = prep_pool.tile([P, NT], f32, name=f"{nm}_f")
        nc.vector.tensor_copy(t_f[:], src_i[:])
        t_b = prep_pool.tile([P, NT], bf16, name=f"{nm}_b")
        nc.vector.tensor_copy(t_b[:], t_f[:])
        nc.sync.dma_start(prep_dram[row].rearrange("(t p) -> p t", p=P), t_b[:])

    cst = prep_pool.tile([1, NB], bf16)
    nc.vector.memset(cst[:], -256.0)
    nc.sync.dma_start(prep_dram[8:9, :], cst[:])
    cst2 = prep_pool.tile([1, NB], bf16)
    nc.vector.memset(cst2[:], -1.0)
    nc.sync.dma_start(prep_dram[9:10, :], cst2[:])

    lhsT_all = row_pool.tile([5, NA], bf16)
    nc.sync.dma_start(lhsT_all[0:3, :], prep_dram[0:3, :])
    nc.sync.dma_start(lhsT_all[3:5, :], prep_dram[8:10, :])
    rhs_all = row_pool.tile([5, NB], bf16)
    nc.sync.dma_start(rhs_all[0:5, :], prep_dram[3:8, :])

    iota_i = row_pool.tile([P, NC], i32)
    nc.gpsimd.iota(iota_i[:], pattern=[[0, K], [1, G]], base=0, channel_multiplier=0)
    iota_f = row_pool.tile([P, NC], f32)
    nc.vector.tensor_copy(iota_f[:], iota_i[:])
    half_c = row_pool.tile([P, 1], f32)
    nc.vector.memset(half_c[:], R)

    qb_blocks = qb_dram.rearrange("(g r) d -> g (r d)", r=G)   # [256, 64] f32

    # ---------------- Phase 1 ----------------
    MM_N = 512
    HALF = 2048
    for t in range(NT):
        gm = sp.tile([P, NG], f32)
        for h in range(2):
            ps = psum_pool.tile([P, HALF], f32)
            for j in range(HALF // MM_N):
                nc.tensor.matmul(
                    out=ps[:, j * MM_N:(j + 1) * MM_N],
                    lhsT=lhsT_all[:, t * P:(t + 1) * P],
                    rhs=rhs_all[:, h * HALF + j * MM_N: h * HALF + (j + 1) * MM_N],
                    start=True, stop=True,
                )
            nc.vector.tensor_reduce(
                out=gm[:, h * (NG // 2):(h + 1) * (NG // 2)],
                in_=ps[:].rearrange("p (g w) -> p g w", w=G),
                axis=X, op=AluOpType.max)

        gv8 = sp.tile([P, 8], f32)
        nc.vector.max(out=gv8[:], in_=gm[:])
        g8 = sp.tile([P, 8], u32)
        nc.vector.max_index(out=g8[:], in_max=gv8[:], in_values=gm[:])

        # gather qb windows: one block-offset per partition per k (u32 offsets)
        qc = sp.tile([P, K, G, 4], f32)
        for k in range(K):
            nc.gpsimd.indirect_dma_start(
                out=qc[:, k].rearrange("p g d -> p (g d)"), out_offset=None,
                in_=qb_blocks,
                in_offset=bass.IndirectOffsetOnAxis(ap=g8[:, k:k + 1], axis=0))

        # exact candidate d2 (POOL)
        qa3 = qa_f[:].rearrange("p (t d) -> p t d", d=3)[:, t, :]
        qcv = qc[:].rearrange("p k g d -> p (k g) d")
        cross = sp.tile([P, NC], f32)
        nc.gpsimd.tensor_scalar(cross[:], qcv[:, :, 0], qa3[:, 0:1], None,
                                op0=AluOpType.mult)
        ty = sp.tile([P, NC], f32)
        nc.gpsimd.tensor_scalar(ty[:], qcv[:, :, 1], qa3[:, 1:2], None,
                                op0=AluOpType.mult)
        nc.gpsimd.tensor_tensor(out=cross[:], in0=cross[:], in1=ty[:],
                                op=AluOpType.add)
        tz = sp.tile([P, NC], f32)
        nc.gpsimd.tensor_scalar(tz[:], qcv[:, :, 2], qa3[:, 2:3], None,
                                op0=AluOpType.mult)
        nc.gpsimd.tensor_tensor(out=cross[:], in0=cross[:], in1=tz[:],
                                op=AluOpType.add)
        d2c = sp.tile([P, NC], f32)
        nc.vector.scalar_tensor_tensor(out=d2c[:], in0=cross[:], scalar=-2.0,
                                       in1=qcv[:, :, 3],
                                       op0=AluOpType.mult, op1=AluOpType.add)
        nc.gpsimd.tensor_scalar(d2c[:], d2c[:], a2[:, t:t + 1], 4094.0,
                                op0=AluOpType.add, op1=AluOpType.min)

        # key48 = -(4096*d2c + m)
        gf3 = sp.tile([P, K], f32)
        nc.gpsimd.tensor_copy(gf3[:], g8[:, 0:K])
        m48 = sp.tile([P, K, G], f32)
        nc.vector.scalar_tensor_tensor(
            out=m48[:],
            in0=gf3[:].rearrange("p (k o) -> p k o", o=1).to_broadcast([P, K, G]),
            scalar=16.0, in1=iota_f[:].rearrange("p (k g) -> p k g", g=G),
            op0=AluOpType.mult, op1=AluOpType.add)
        key48 = sp.tile([P, NC], f32)
        nc.vector.scalar_tensor_tensor(out=key48[:], in0=d2c[:], scalar=-4096.0,
                                       in1=m48[:].rearrange("p k g -> p (k g)"),
                                       op0=AluOpType.mult, op1=AluOpType.subtract)

        kv8 = sp.tile([P, 8], f32)
        nc.vector.max(out=kv8[:], in_=key48[:])

        # decode (POOL): r = -key ; m = r & 4095 ; d2 = r >> 12
        r3i = sp.tile([P, K], i32)
        nc.gpsimd.tensor_scalar(r3i[:], kv8[:, 0:K], -1.0, None, op0=AluOpType.mult)
        m3 = sp.tile([P, K], i32)
        nc.vector.tensor_scalar(m3[:], r3i[:], 4095, None, op0=AluOpType.bitwise_and)
        d2_3i = sp.tile([P, K], i32)
        nc.vector.tensor_scalar(d2_3i[:], r3i[:], 12, None,
                                op0=AluOpType.arith_shift_right)
        d2_3f = sp.tile([P, K], f32)
        nc.gpsimd.tensor_copy(d2_3f[:], d2_3i[:])

        # weights: w = relu(0.5 - sqrt(d2)/128)  (exact: 0.5 - clip(dist,0,.5))
        s3 = sp.tile([P, K], f32)
        nc.scalar.activation(s3[:], d2_3f[:], Act.Sqrt)
        w3 = sp.tile([P, K], f32)
        nc.scalar.activation(w3[:], s3[:], Act.Relu, scale=-1.0 / FS, bias=half_c[:])

        # feature gather: one row per partition per k
        gath = sp.tile([P, K, D], f32)
        for k in range(K):
            nc.gpsimd.indirect_dma_start(
                out=gath[:, k, :], out_offset=None, in_=b_feats[:],
                in_offset=bass.IndirectOffsetOnAxis(ap=m3[:, k:k + 1], axis=0))

        mk = [sp.tile([P, D], f32, name=f"mk{k}_{t}", tag=f"mk{k}")
              for k in range(K)]
        for k in range(K):
            nc.scalar.activation(mk[k][:], gath[:, k, :], Act.Copy,
                                 scale=w3[:, k:k + 1])
        acc = sp.tile([P, D], f32)
        nc.vector.tensor_tensor(out=acc[:], in0=mk[0][:], in1=mk[1][:],
                                op=AluOpType.add)
        nc.vector.tensor_tensor(out=acc[:], in0=acc[:], in1=mk[2][:],
                                op=AluOpType.add)
        nc.sync.dma_start(out[t * P:(t + 1) * P, D:2 * D], acc[:])

    nc.sync.dma_start(out[:, 0:D], a_feats[:])


B = 8
_PROGRAM_CACHE = {}


def build_program():
    if "nc" in _PROGRAM_CACHE:
        return _PROGRAM_CACHE["nc"]
    nc = bacc.Bacc("TRN2", target_bir_lowering=False, debug=False)
    a_feats = nc.dram_tensor("a_feats", [NA, D], f32, kind="ExternalInput").ap()
    b_feats = nc.dram_tensor("b_feats", [NB, D], f32, kind="ExternalInput").ap()
    a_coords = nc.dram_tensor("a_coords", [NA, 3], i32, kind="ExternalInput").ap()
    b_coords = nc.dram_tensor("b_coords", [NB, 3], i32, kind="ExternalInput").ap()
    out = nc.dram_tensor("out", [NA, 2 * D], f32, kind="ExternalOutput").ap()
    with tile.TileContext(nc) as tc:
        knn_tile_kernel3(tc, [out], [a_feats, b_feats, a_coords, b_coords])
    nc.compile()
    _PROGRAM_CACHE["nc"] = nc
    return nc


def kernel(a_feats, b_feats, a_coords, b_coords, _trace=False):
    nc = build_program()
    in_maps = [
        {
            "a_feats": np.ascontiguousarray(a_feats[b], dtype=np.float32),
            "b_feats": np.ascontiguousarray(b_feats[b], dtype=np.float32),
            "a_coords": np.ascontiguousarray(a_coords[b], dtype=np.int32),
            "b_coords": np.ascontiguousarray(b_coords[b], dtype=np.int32),
        }
        for b in range(B)
    ]
    res = run_bass_kernel_spmd(nc, in_maps, list(range(B)), trace=_trace)
    out = np.stack([np.asarray(res.results[b]["out"]) for b in range(B)], axis=0)
    if _trace:
        return out.astype(np.float32), res
    return out.astype(np.float32)



# revision 11
# speedup vs baseline: 1.0107x; 1.0107x over previous
"""Trainium2 Bass kernel for retrieval_knn (nn_CLI_63702954934481).

Data-parallel over batch B=8: one batch per NeuronCore. Per core:
7-row bf16 PE matmul computes key = -d2 exactly in PSUM f32 (a^2 and b^2
split into bf16-exact byte halves). Selection: pairwise tensor_tensor max
tree (round 1 f32 PSUM -> bf16 SBUF, later rounds bf16 at 2x DVE rate),
hw top-8 max/max_index over group maxima, exact re-rank of the top NSEL
groups' candidates with full (d2, index) tie-breaking, indirect-DMA
gathers, weighted sum, concat. Self-contained.
"""
import sys

if "/opt/trn_rl_repo" not in sys.path:
    sys.path.insert(0, "/opt/trn_rl_repo")

import numpy as np
from concourse import bacc
from concourse.bass_utils import run_bass_kernel_spmd

from contextlib import ExitStack

import concourse.bass as bass
import concourse.mybir as mybir
import concourse.tile as tile
from concourse._compat import with_exitstack
from concourse.alu_op_type import AluOpType

P = 128
NA = 4096
NB = 4096
D = 64
K = 3
NT = NA // P      # 32 a-tiles
G = 16            # group size along b
NG = NB // G      # 256 groups
NSEL = 4          # groups re-ranked per a-row
NC = NSEL * G     # 64 candidates
FS = 128.0
R = 0.5

MM_N = 512        # matmul chunk (one PSUM bank)
HALF = 2048       # b-columns per PSUM tile
HG = HALF // G    # 128 groups per half
ND = 48           # groups per half drained by DVE tensor_reduce
CA = HG - ND      # groups per half converted to bf16 by Act (Pool can't
CT = CA           # read PSUM; the drain is DVE+Act only)

f32 = mybir.dt.float32
i32 = mybir.dt.int32
u32 = mybir.dt.uint32
bf16 = mybir.dt.bfloat16
Act = mybir.ActivationFunctionType
X = mybir.AxisListType.X


@with_exitstack
def knn_kernel(ctx: ExitStack, tc: tile.TileContext, outs, ins):
    nc = tc.nc
    a_feats, b_feats, a_coords, b_coords = ins
    out = outs[0]

    prep_pool = ctx.enter_context(tc.tile_pool(name="prep", bufs=1))
    row_pool = ctx.enter_context(tc.tile_pool(name="rows", bufs=1))
    psum_pool = ctx.enter_context(tc.tile_pool(name="ps", bufs=2, space="PSUM"))
    sp = ctx.enter_context(tc.tile_pool(name="small", bufs=4))

    # ---------------- Phase 0: prep ----------------
    # lhsT rows (per a): [2ax, 2ay, 2az, ah, al, 1, 1]
    # rhs rows (per b):  [bx, by, bz, -256, -1, -256*bh, -bl]
    # PSUM key = 2a.b - a^2 - b^2 = -d2 (exact f32 integer)
    prep_dram = nc.dram_tensor("prep_scratch", [14, NB], bf16)
    qb_dram = nc.dram_tensor("qb_scratch", [NB, 4], f32)

    def load_q(coords, tag):
        c = prep_pool.tile([P, NT * 3], i32, name=f"c_{tag}")
        nc.sync.dma_start(c[:], coords.rearrange("(t p) d -> p t d", p=P))
        q_i = prep_pool.tile([P, NT * 3], i32, name=f"qi_{tag}")
        nc.vector.tensor_scalar(q_i[:], c[:], 4, None, op0=AluOpType.arith_shift_right)
        q_f = prep_pool.tile([P, NT * 3], f32, name=f"qf_{tag}")
        nc.vector.tensor_copy(q_f[:], q_i[:])
        return q_f

    def sq_sum(q_f, tag):
        sq = prep_pool.tile([P, NT * 3], f32, name=f"sq_{tag}")
        nc.vector.tensor_tensor(out=sq[:], in0=q_f[:], in1=q_f[:], op=AluOpType.mult)
        s2 = prep_pool.tile([P, NT], f32, name=f"s2_{tag}")
        nc.vector.tensor_reduce(out=s2[:], in_=sq[:].rearrange("p (t d) -> p t d", d=3),
                                axis=X, op=AluOpType.add)
        return s2

    def split_bytes(s2, tag):
        # s2 integer-valued f32 in [0, 49152): hi = s2>>8, lo = s2&255 (f32)
        s2_i = prep_pool.tile([P, NT], i32, name=f"s2i_{tag}")
        nc.vector.tensor_copy(s2_i[:], s2[:])
        hi_i = prep_pool.tile([P, NT], i32, name=f"hi_{tag}")
        nc.vector.tensor_scalar(hi_i[:], s2_i[:], 8, None, op0=AluOpType.arith_shift_right)
        lo_i = prep_pool.tile([P, NT], i32, name=f"lo_{tag}")
        nc.vector.tensor_scalar(lo_i[:], s2_i[:], 255, None, op0=AluOpType.bitwise_and)
        hi_f = prep_pool.tile([P, NT], f32, name=f"hif_{tag}")
        nc.vector.tensor_copy(hi_f[:], hi_i[:])
        lo_f = prep_pool.tile([P, NT], f32, name=f"lof_{tag}")
        nc.vector.tensor_copy(lo_f[:], lo_i[:])
        return hi_f, lo_f

    def to_rows(vals_scales, base_row):
        # vals_scales: list of (tile_f32 [P, NT*k] viewed (t d), inner_k,
        # d_select, scale); each produces one prep_dram row of NB values
        for i, (src_f, kk, dsel, scale) in enumerate(vals_scales):
            t_b = prep_pool.tile([P, NT], bf16, name=f"row{base_row + i}")
            if kk == 1:
                view = src_f[:]
            else:
                view = src_f[:].rearrange("p (t d) -> p t d", d=kk)[:, :, dsel]
            nc.vector.tensor_scalar(t_b[:], view, scale, None, op0=AluOpType.mult)
            nc.sync.dma_start(prep_dram[base_row + i].rearrange("(t p) -> p t", p=P),
                              t_b[:])

    qa_f = load_q(a_coords, "a")
    qb_f = load_q(b_coords, "b")
    a2 = sq_sum(qa_f, "a")
    b2 = sq_sum(qb_f, "b")
    ah, al = split_bytes(a2, "a")
    bh, bl = split_bytes(b2, "b")

    # lhsT source rows 0..4: 2ax,2ay,2az, ah, al
    to_rows([(qa_f, 3, 0, 2.0), (qa_f, 3, 1, 2.0), (qa_f, 3, 2, 2.0),
             (ah, 1, 0, 1.0), (al, 1, 0, 1.0)], 0)
    # rhs source rows 5..9: bx,by,bz, -256*bh, -bl
    to_rows([(qb_f, 3, 0, 1.0), (qb_f, 3, 1, 1.0), (qb_f, 3, 2, 1.0),
             (bh, 1, 0, -256.0), (bl, 1, 0, -1.0)], 5)

    # qb_blocks for exact re-rank: (bx, by, bz, b2) f32 per b point
    nc.sync.dma_start(
        qb_dram.rearrange("(t p) d -> p t d", p=P)[:, :, 0:3],
        qb_f[:].rearrange("p (t d) -> p t d", d=3))
    nc.sync.dma_start(qb_dram.rearrange("(t p) d -> p t d", p=P)[:, :, 3], b2[:])

    # constant rows for lhsT/rhs
    ones_c = prep_pool.tile([1, NB], bf16)
    nc.vector.memset(ones_c[:], 1.0)
    nc.sync.dma_start(prep_dram[10:11, :], ones_c[:])
    n256_c = prep_pool.tile([1, NB], bf16)
    nc.vector.memset(n256_c[:], -256.0)
    nc.sync.dma_start(prep_dram[11:12, :], n256_c[:])
    n1_c = prep_pool.tile([1, NB], bf16)
    nc.vector.memset(n1_c[:], -1.0)
    nc.sync.dma_start(prep_dram[12:13, :], n1_c[:])

    # assemble matmul operands in SBUF
    lhsT_all = row_pool.tile([7, NA], bf16)     # [2ax,2ay,2az,ah,al,1,1]
    nc.sync.dma_start(lhsT_all[0:5, :], prep_dram[0:5, :])
    nc.sync.dma_start(lhsT_all[5:7, :], prep_dram[10:11, :].to_broadcast([2, NA]))
    rhs_all = row_pool.tile([7, NB], bf16)      # [bx,by,bz,-256,-1,-256bh,-bl]
    nc.sync.dma_start(rhs_all[0:3, :], prep_dram[5:8, :])
    nc.sync.dma_start(rhs_all[3:4, :], prep_dram[11:12, :])
    nc.sync.dma_start(rhs_all[4:5, :], prep_dram[12:13, :])
    nc.sync.dma_start(rhs_all[5:7, :], prep_dram[8:10, :])

    # iotas
    iota16_i = row_pool.tile([P, NC], i32)      # j within group, per candidate
    nc.gpsimd.iota(iota16_i[:], pattern=[[0, NSEL], [1, G]], base=0,
                   channel_multiplier=0)
    iota16 = row_pool.tile([P, NC], f32)
    nc.vector.tensor_copy(iota16[:], iota16_i[:])
    half_c = row_pool.tile([P, 1], f32)
    nc.vector.memset(half_c[:], R)

    # gm-column -> logical-group-id table (gm layout: [DVE h0 | DVE h1 |
    # tree h0 | tree h1] where DVE covers the first ND groups of each half)
    gt_i = row_pool.tile([P, NG], i32)
    nc.gpsimd.iota(gt_i[:, 0:ND], pattern=[[1, ND]], base=0, channel_multiplier=0)
    nc.gpsimd.iota(gt_i[:, ND:2 * ND], pattern=[[1, ND]], base=HG,
                   channel_multiplier=0)
    nc.gpsimd.iota(gt_i[:, 2 * ND:2 * ND + CT], pattern=[[1, CT]], base=ND,
                   channel_multiplier=0)
    nc.gpsimd.iota(gt_i[:, 2 * ND + CT:NG], pattern=[[1, CT]], base=HG + ND,
                   channel_multiplier=0)
    gtable = row_pool.tile([P, NG], f32)
    nc.vector.tensor_copy(gtable[:], gt_i[:])

    qb_blocks = qb_dram.rearrange("(g r) d -> g (r d)", r=G)   # [256, 64] f32

    # ---------------- Phase 1: per-tile pipeline ----------------
    for t in range(NT):
        # PSUM key drain, 3-way: DVE reduces first ND groups of each half;
        # Act converts next CA groups, Pool converts last CP groups to bf16;
        # DVE runs the pairwise-max tree over the converted slab.
        gm = sp.tile([P, NG], bf16, name=f"gm_{t}", tag="gm")
        gmf = sp.tile([P, 2, ND], f32, name=f"gmf_{t}", tag="gmf")
        kb = sp.tile([P, 2, CT, G], bf16, name=f"kb_{t}", tag="kb")
        for h in range(2):
            ps = psum_pool.tile([P, HALF], f32, name=f"ps_{t}_{h}", tag="ps")
            for j in range(HALF // MM_N):
                nc.tensor.matmul(
                    out=ps[:, j * MM_N:(j + 1) * MM_N],
                    lhsT=lhsT_all[:, t * P:(t + 1) * P],
                    rhs=rhs_all[:, h * HALF + j * MM_N: h * HALF + (j + 1) * MM_N],
                    start=True, stop=True,
                )
            psv = ps[:].rearrange("p (g w) -> p g w", w=G)
            nc.vector.tensor_reduce(out=gmf[:, h, :], in_=psv[:, 0:ND, :],
                                    axis=X, op=AluOpType.max)
            nc.scalar.activation(kb[:, h, 0:CA, :],
                                 psv[:, ND:ND + CA, :].rearrange("p g w -> p (g w)"),
                                 Act.Copy)
        # DVE-part group maxima -> gm[:, 0:2*ND]
        nc.vector.tensor_copy(gm[:, 0:2 * ND], gmf[:].rearrange("p h g -> p (h g)"))
        # bf16 pairwise-max tree over converted slab -> gm[:, 2*ND:]
        kbv = kb[:].rearrange("p h g w -> p (h g) w")
        r1 = sp.tile([P, 2 * CT, 8], bf16, name=f"r1_{t}", tag="r1")
        nc.vector.tensor_tensor(out=r1[:], in0=kbv[:, :, 0:8], in1=kbv[:, :, 8:16],
                                op=AluOpType.max)
        r2 = sp.tile([P, 2 * CT, 4], bf16, name=f"r2_{t}", tag="r2")
        nc.vector.tensor_tensor(out=r2[:], in0=r1[:, :, 0:4], in1=r1[:, :, 4:8],
                                op=AluOpType.max)
        r3 = sp.tile([P, 2 * CT, 2], bf16, name=f"r3_{t}", tag="r3")
        nc.vector.tensor_tensor(out=r3[:], in0=r2[:, :, 0:2], in1=r2[:, :, 2:4],
                                op=AluOpType.max)
        nc.vector.tensor_tensor(
            out=gm[:, 2 * ND:NG].rearrange("p (g w) -> p g w", w=1),
            in0=r3[:, :, 0:1], in1=r3[:, :, 1:2], op=AluOpType.max)

        # embed logical group id: gm_g = 256*gm - gtable (f32, 24-bit exact)
        gm_g = sp.tile([P, NG], f32, name=f"gmg_{t}", tag="gmg")
        nc.vector.scalar_tensor_tensor(out=gm_g[:], in0=gm[:], scalar=256.0,
                                       in1=gtable[:], op0=AluOpType.mult,
                                       op1=AluOpType.subtract)
        gv8 = sp.tile([P, 8], f32, name=f"gv8_{t}", tag="gv8")
        nc.vector.max(out=gv8[:], in_=gm_g[:])
        # decode group ids of top NSEL: g = (-v) & 255
        rg = sp.tile([P, NSEL], i32, name=f"rg_{t}", tag="rg")
        nc.gpsimd.tensor_scalar(rg[:], gv8[:, 0:NSEL], -1.0, None, op0=AluOpType.mult)
        g8 = sp.tile([P, NSEL], i32, name=f"g8_{t}", tag="g8")
        nc.vector.tensor_scalar(g8[:], rg[:], 255, None, op0=AluOpType.bitwise_and)

        # gather qb windows for top NSEL groups (one indirect DMA)
        qc = sp.tile([P, NSEL, G, 4], f32, name=f"qc_{t}", tag="qc")
        nc.gpsimd.indirect_dma_start(
            out=qc[:].rearrange("p s g d -> p (s g d)"), out_offset=None,
            in_=qb_blocks,
            in_offset=bass.IndirectOffsetOnAxis(ap=g8[:], axis=0))

        # exact candidate d2 (Act does the 3 coordinate products)
        qa3 = qa_f[:].rearrange("p (t d) -> p t d", d=3)[:, t, :]
        qcv = qc[:].rearrange("p s g d -> p (s g) d")
        m0 = sp.tile([P, NC], f32, name=f"m0_{t}", tag="m0")
        nc.scalar.activation(m0[:], qcv[:, :, 0], Act.Copy, scale=qa3[:, 0:1])
        m1 = sp.tile([P, NC], f32, name=f"m1_{t}", tag="m1")
        nc.scalar.activation(m1[:], qcv[:, :, 1], Act.Copy, scale=qa3[:, 1:2])
        m2 = sp.tile([P, NC], f32, name=f"m2_{t}", tag="m2")
        nc.scalar.activation(m2[:], qcv[:, :, 2], Act.Copy, scale=qa3[:, 2:3])
        cross = sp.tile([P, NC], f32, name=f"cr_{t}", tag="cr")
        nc.gpsimd.tensor_tensor(out=cross[:], in0=m0[:], in1=m1[:], op=AluOpType.add)
        nc.gpsimd.tensor_tensor(out=cross[:], in0=cross[:], in1=m2[:], op=AluOpType.add)
        d2c = sp.tile([P, NC], f32, name=f"d2c_{t}", tag="d2c")
        nc.vector.scalar_tensor_tensor(out=d2c[:], in0=cross[:], scalar=-2.0,
                                       in1=qcv[:, :, 3],
                                       op0=AluOpType.mult, op1=AluOpType.add)
        nc.gpsimd.tensor_scalar(d2c[:], d2c[:], a2[:, t:t + 1], 4094.0,
                                op0=AluOpType.add, op1=AluOpType.min)

        # key24 = -(4096*d2 + idx); idx = 16*g + j
        g8f = sp.tile([P, NSEL], f32, name=f"g8f_{t}", tag="g8f")
        nc.gpsimd.tensor_copy(g8f[:], g8[:])
        idxf = sp.tile([P, NSEL, G], f32, name=f"idxf_{t}", tag="idxf")
        nc.vector.scalar_tensor_tensor(
            out=idxf[:],
            in0=g8f[:].rearrange("p (s o) -> p s o", o=1).to_broadcast([P, NSEL, G]),
            scalar=16.0, in1=iota16[:].rearrange("p (s g) -> p s g", g=G),
            op0=AluOpType.mult, op1=AluOpType.add)
        key24 = sp.tile([P, NC], f32, name=f"k24_{t}", tag="k24")
        nc.vector.scalar_tensor_tensor(out=key24[:], in0=d2c[:], scalar=-4096.0,
                                       in1=idxf[:].rearrange("p s g -> p (s g)"),
                                       op0=AluOpType.mult, op1=AluOpType.subtract)

        kv8 = sp.tile([P, 8], f32, name=f"kv8_{t}", tag="kv8")
        nc.vector.max(out=kv8[:], in_=key24[:])

        # decode top-3: r = -key; m = r & 4095; d2 = r >> 12
        r3i = sp.tile([P, K], i32, name=f"r3i_{t}", tag="r3i")
        nc.gpsimd.tensor_scalar(r3i[:], kv8[:, 0:K], -1.0, None, op0=AluOpType.mult)
        m3 = sp.tile([P, K], i32, name=f"m3_{t}", tag="m3")
        nc.vector.tensor_scalar(m3[:], r3i[:], 4095, None, op0=AluOpType.bitwise_and)
        d2_3i = sp.tile([P, K], i32, name=f"d23i_{t}", tag="d23i")
        nc.vector.tensor_scalar(d2_3i[:], r3i[:], 12, None,
                                op0=AluOpType.arith_shift_right)
        d2_3f = sp.tile([P, K], f32, name=f"d23f_{t}", tag="d23f")
        nc.gpsimd.tensor_copy(d2_3f[:], d2_3i[:])

        # weights: w = relu(0.5 - sqrt(d2)/128)
        s3 = sp.tile([P, K], f32, name=f"s3_{t}", tag="s3")
        nc.scalar.activation(s3[:], d2_3f[:], Act.Sqrt)
        w3 = sp.tile([P, K], f32, name=f"w3_{t}", tag="w3")
        nc.scalar.activation(w3[:], s3[:], Act.Relu, scale=-1.0 / FS, bias=half_c[:])

        # feature gather (one indirect DMA for all K)
        gath = sp.tile([P, K, D], f32, name=f"ga_{t}", tag="ga")
        nc.gpsimd.indirect_dma_start(
            out=gath[:].rearrange("p k d -> p (k d)"), out_offset=None,
            in_=b_feats[:],
            in_offset=bass.IndirectOffsetOnAxis(ap=m3[:, 0:K], axis=0))

        # weighted sum: Act multiplies, Pool adds
        mk = [sp.tile([P, D], f32, name=f"mk{k}_{t}", tag=f"mk{k}") for k in range(K)]
        for k in range(K):
            nc.scalar.activation(mk[k][:], gath[:, k, :], Act.Copy,
                                 scale=w3[:, k:k + 1])
        acc = sp.tile([P, D], f32, name=f"acc_{t}", tag="acc")
        nc.gpsimd.tensor_tensor(out=acc[:], in0=mk[0][:], in1=mk[1][:],
                                op=AluOpType.add)
        nc.gpsimd.tensor_tensor(out=acc[:], in0=acc[:], in1=mk[2][:],
                                op=AluOpType.add)
        nc.sync.dma_start(out[t * P:(t + 1) * P, D:2 * D], acc[:])

    nc.sync.dma_start(out[:, 0:D], a_feats[:])


B = 8
_PROGRAM_CACHE = {}


def build_program():
    if "nc" in _PROGRAM_CACHE:
        return _PROGRAM_CACHE["nc"]
    nc = bacc.Bacc("TRN2", target_bir_lowering=False, debug=False)
    a_feats = nc.dram_tensor("a_feats", [NA, D], f32, kind="ExternalInput").ap()
    b_feats = nc.dram_tensor("b_feats", [NB, D], f32, kind="ExternalInput").ap()
    a_coords = nc.dram_tensor("a_coords", [NA, 3], i32, kind="ExternalInput").ap()
    b_coords = nc.dram_tensor("b_coords", [NB, 3], i32, kind="ExternalInput").ap()
    out = nc.dram_tensor("out", [NA, 2 * D], f32, kind="ExternalOutput").ap()
    with tile.TileContext(nc) as tc:
        knn_kernel(tc, [out], [a_feats, b_feats, a_coords, b_coords])
    nc.compile()
    _PROGRAM_CACHE["nc"] = nc
    return nc


def kernel(a_feats, b_feats, a_coords, b_coords, _trace=False):
    nc = build_program()
    in_maps = [
        {
            "a_feats": np.ascontiguousarray(a_feats[b], dtype=np.float32),
            "b_feats": np.ascontiguousarray(b_feats[b], dtype=np.float32),
            "a_coords": np.ascontiguousarray(a_coords[b], dtype=np.int32),
            "b_coords": np.ascontiguousarray(b_coords[b], dtype=np.int32),
        }
        for b in range(B)
    ]
    res = run_bass_kernel_spmd(nc, in_maps, list(range(B)), trace=_trace)
    out = np.stack([np.asarray(res.results[b]["out"]) for b in range(B)], axis=0)
    if _trace:
        return out.astype(np.float32), res
    return out.astype(np.float32)
